# revision 23
# baseline (speedup 1.0000x reference)
"""Trainium2 Bass kernel for nn_AttentionUnit (multi-head attention block), v2.

Reference math (B=2, S=2048, D=1024, H=16 heads, d_head=64, fp32):
    Q = q @ wq.T + bq ; K = k @ wk.T + bk ; V = v @ wv.T + bv
    S = QK^T / 8  (per head), causal mask + key-padding mask
    out = softmax(S) @ V  -> concat heads -> @ wo.T + bo

Sharding (8 cores): data-parallel over batch (2 groups of 4 cores),
tensor-parallel over heads (4 heads/core).  Column-parallel QKV,
row-parallel wo with per-q-block in-group ReduceScatter(add).

v2 structure (single software-pipelined stream):
  - Projections are interleaved with attention per 512-wide panel:
    K_p/V_p/Q_p proj for panel p are emitted around attention block p-1,
    so the PE starts ~4us in and the scalar-engine exp window spans the
    whole run instead of just the attention phase.
  - Host packs x^T as [128, 8, 2048] and w^T as [128, 8, 256] so one DMA
    loads a full 512-column panel (12 x-DMAs total); no 1-row DMAs.
  - Q/K biases are added during PSUM eviction (tensor_scalar add with a
    per-partition bias column); V bias is folded into bo4 on the host
    (exact: softmax weights sum to 1 after normalization); the key
    padding mask is folded into V rows and the row-sum ones-column
    (padded keys contribute 0 to both numerator and denominator).
  - Scores land transposed (S^T[k, q]) in paired-k-tile PSUM tiles
    [128, 1024] (2 banks) so one exp call covers 2 k-tiles; softmax max
    subtraction is skipped (scores are O(1)); causal masking multiplies
    only the [128,128] diagonal triangle of each diagonal tile.
  - ctx normalization is a per-partition reciprocal+scale; head pairs
    share one [128,128] transpose back to ctx^T layout.
  - Out-proj eviction (+bo) on vector, partial-out DMAs on sync,
    per-q-block ReduceScatter on gpsimd (which must otherwise stay clear:
    a collective blocks the Pool queue for its full duration); the host
    reassembles the chunk-major outputs.
"""

import os
import sys
from contextlib import ExitStack

import numpy as np

try:
    import concourse.bass as bass
except ImportError:  # harness containers keep the repo at /opt/trn_rl_repo
    for _p in ("/opt/trn_rl_repo", "/root/.axon_site/_ro/trn_rl_repo"):
        if os.path.isdir(_p) and _p not in sys.path:
            sys.path.insert(0, _p)
    import concourse.bass as bass

from concourse import bacc

import ml_dtypes
import concourse.mybir as mybir
import concourse.tile as tile
from concourse.bass_utils import run_bass_kernel_spmd

BF16 = ml_dtypes.bfloat16

B = 2
SEQ = 2048
D = 1024
H = 16
DH = 64
NCORES = 8
G = 4            # tensor-parallel group size (cores per batch)
HPC = H // G     # heads per core
DPC = HPC * DH   # head dims per core (256)
QB = 512         # q block width / panel width
KT = 128         # k tile height
NEG = -30000.0


def build_program(seq=SEQ, d=D, hpc=HPC):
    """Emit the SPMD program (identical on all 8 cores)."""
    fp32 = mybir.dt.float32
    bf16 = mybir.dt.bfloat16
    dpc = hpc * DH
    n_qb = seq // QB          # 4 panels / q blocks
    n_kt = seq // KT          # 16 k tiles
    n_dt = d // 128           # 8 contraction tiles of the model dim
    n_mt = dpc // 128         # 2 128-row tiles of the per-core head dims
    sub = QB // KT            # k-tiles per q block on the diagonal (4)

    nc = bacc.Bacc(num_devices=NCORES)

    # packed inputs (see prep_core_inputs)
    xq_d = nc.declare_dram_parameter("xq", [128, n_dt, seq], bf16, False)
    xk_d = nc.declare_dram_parameter("xk", [128, n_dt, seq], bf16, False)
    xv_d = nc.declare_dram_parameter("xv", [128, n_dt, seq], bf16, False)
    wq_d = nc.declare_dram_parameter("wq", [128, n_dt, dpc], bf16, False)
    wk_d = nc.declare_dram_parameter("wk", [128, n_dt, dpc], bf16, False)
    wv_d = nc.declare_dram_parameter("wv", [128, n_dt, dpc], bf16, False)
    woT = nc.declare_dram_parameter("woT", [dpc, d], bf16, False)
    tri_d = nc.declare_dram_parameter("tri", [KT, KT], bf16, False)
    ident_d = nc.declare_dram_parameter("ident", [128, 128], bf16, False)
    bo4_d = nc.declare_dram_parameter("bo4", [128, d], fp32, False)
    bq_d = nc.declare_dram_parameter("bq", [128, n_mt], fp32, False)
    bk_d = nc.declare_dram_parameter("bk", [128, n_mt], fp32, False)
    vmask_d = nc.declare_dram_parameter("vmask", [128, n_kt], fp32, False)
    out_ext = nc.declare_dram_parameter("out", [n_qb, 128, d], bf16, isOutput=True)

    # one partial tensor per q block: RS(qb) reading must not serialize
    # against qb+1's partial writes (WAR on a shared tensor)
    partials = [nc.dram_tensor(f"partial{c}", [QB, d], bf16) for c in range(seq // QB)]
    rs_out = nc.dram_tensor("rs_out", [n_qb, 128, d], bf16)

    groups = [[0, 1, 2, 3], [4, 5, 6, 7]]

    with ExitStack() as ctx:
        tc = ctx.enter_context(tile.TileContext(nc, num_cores=NCORES))

        xqp = ctx.enter_context(tc.tile_pool(name="xqp", bufs=2))
        xkvp = ctx.enter_context(tc.tile_pool(name="xkvp", bufs=1))
        persist = ctx.enter_context(tc.tile_pool(name="persist", bufs=1))
        vpool = ctx.enter_context(tc.tile_pool(name="vp", bufs=1))
        ppool = ctx.enter_context(tc.tile_pool(name="pp", bufs=36))
        spool = ctx.enter_context(tc.tile_pool(name="sp", bufs=8))
        opool = ctx.enter_context(tc.tile_pool(name="op", bufs=4))
        cqpool = ctx.enter_context(tc.tile_pool(name="cq", bufs=4))
        osbp = ctx.enter_context(tc.tile_pool(name="osb", bufs=2))
        psS = ctx.enter_context(tc.tile_pool(name="psS", bufs=2, space="PSUM"))
        psC = ctx.enter_context(tc.tile_pool(name="psC", bufs=2, space="PSUM"))
        psD = ctx.enter_context(tc.tile_pool(name="psD", bufs=2, space="PSUM"))

        # ---- persistent activation tiles ----
        QT = [persist.tile([64, seq], bf16, tag=f"QT{h}", name=f"QT{h}") for h in range(hpc)]
        KTt = [persist.tile([64, seq], bf16, tag=f"KT{h}", name=f"KT{h}") for h in range(hpc)]
        V_sb = [vpool.tile([128, hpc, 65], bf16, tag=f"V{m}", name=f"V{m}") for m in range(n_kt)]
        ctxT = [persist.tile([128, seq], bf16, tag=f"ctxT{t}", name=f"ctxT{t}") for t in range(n_mt)]

        # ---- weights + x panels + constants ----
        # Only two queues carry input DMAs (SP for k/v-side, ACT for q-side)
        # so the shared DMA engines serve the critical path (wk, xk0, wq,
        # xq0) first; wv/xv ride behind xk on SP, constants tail.  The
        # first xk/xq panels are split so projections start on half data.
        wk_sb = persist.tile([128, n_dt, dpc], bf16, tag="wk")
        nc.sync.dma_start(out=wk_sb[:, 0:4, :], in_=wk_d[:, 0:4, :])
        nc.sync.dma_start(out=wk_sb[:, 4:8, :], in_=wk_d[:, 4:8, :])
        wq_sb = persist.tile([128, n_dt, dpc], bf16, tag="wq")
        nc.scalar.dma_start(out=wq_sb[:, 0:4, :], in_=wq_d[:, 0:4, :])
        nc.scalar.dma_start(out=wq_sb[:, 4:8, :], in_=wq_d[:, 4:8, :])

        xk_t = {}
        xv_t = {}
        xq_t = {}
        xk_t[0] = xkvp.tile([128, n_dt, QB], bf16, tag="xk", name="xk0")
        nc.sync.dma_start(out=xk_t[0][:, 0:4, :], in_=xk_d[:, 0:4, 0:QB])
        nc.sync.dma_start(out=xk_t[0][:, 4:8, :], in_=xk_d[:, 4:8, 0:QB])
        xq_t[0] = xqp.tile([128, n_dt, QB], bf16, tag="xq", name="xq0")
        nc.scalar.dma_start(out=xq_t[0][:, 0:4, :], in_=xq_d[:, 0:4, 0:QB])
        nc.scalar.dma_start(out=xq_t[0][:, 4:8, :], in_=xq_d[:, 4:8, 0:QB])

        wv_sb = persist.tile([128, n_dt, dpc], bf16, tag="wv")
        nc.sync.dma_start(out=wv_sb, in_=wv_d[:, :, :])
        # tiny constants ride the otherwise-idle Pool queue so their issue
        # time doesn't delay the xk/xv panel stream on SP (collectives
        # don't claim Pool until ~45us in)
        vmask_sb = persist.tile([128, n_kt], fp32, tag="vmask")
        nc.gpsimd.dma_start(out=vmask_sb, in_=vmask_d[:, :])
        bq_sb = persist.tile([128, n_mt], fp32, tag="bq")
        nc.gpsimd.dma_start(out=bq_sb, in_=bq_d[:, :])
        bk_sb = persist.tile([128, n_mt], fp32, tag="bk")
        nc.gpsimd.dma_start(out=bk_sb, in_=bk_d[:, :])
        tri_sb = persist.tile([KT, KT], bf16, tag="tri")
        nc.gpsimd.dma_start(out=tri_sb, in_=tri_d[:, :])

        xk_t[1] = xkvp.tile([128, n_dt, QB], bf16, tag="xk", name="xk1")
        nc.sync.dma_start(out=xk_t[1], in_=xk_d[:, :, QB:2 * QB])
        for p in range(1, n_qb):
            xq_t[p] = xqp.tile([128, n_dt, QB], bf16, tag="xq", name=f"xq{p}")
            nc.scalar.dma_start(out=xq_t[p], in_=xq_d[:, :, p * QB:(p + 1) * QB])
        xv_t[0] = xkvp.tile([128, n_dt, QB], bf16, tag="xv", name="xv0")
        nc.sync.dma_start(out=xv_t[0], in_=xv_d[:, :, 0:QB])
        for p in range(2, n_qb):
            xk_t[p] = xkvp.tile([128, n_dt, QB], bf16, tag="xk", name=f"xk{p}")
            nc.sync.dma_start(out=xk_t[p], in_=xk_d[:, :, p * QB:(p + 1) * QB])
        for p in range(1, n_qb):
            xv_t[p] = xkvp.tile([128, n_dt, QB], bf16, tag="xv", name=f"xv{p}")
            nc.sync.dma_start(out=xv_t[p], in_=xv_d[:, :, p * QB:(p + 1) * QB])

        ident_sb = persist.tile([128, 128], bf16, tag="ident")
        nc.sync.dma_start(out=ident_sb, in_=ident_d[:, :])
        wo_sb = [persist.tile([128, d], bf16, tag=f"wo{t}", name=f"wo{t}") for t in range(n_mt)]
        for t in range(n_mt):
            nc.sync.dma_start(out=wo_sb[t], in_=woT[t * 128:(t + 1) * 128, :])
        bo4_sb = persist.tile([128, d], fp32, tag="bo4")
        nc.sync.dma_start(out=bo4_sb, in_=bo4_d[:, :])

        # ---- pipeline building blocks ----
        def k_proj(p):
            """K^T for keys [p*QB, (p+1)*QB) -> KTt[h][:, panel]; bias via DVE."""
            for mt in range(n_mt):
                ps = psD.tile([128, QB], fp32, tag="proj")
                for kti in range(n_dt):
                    nc.tensor.matmul(
                        out=ps,
                        lhsT=wk_sb[:, kti, mt * 128:(mt + 1) * 128],
                        rhs=xk_t[p][:, kti, :],
                        start=(kti == 0),
                        stop=(kti == n_dt - 1),
                    )
                for hl in range(2):
                    h = 2 * mt + hl
                    nc.vector.tensor_scalar(
                        out=KTt[h][0:64, p * QB:(p + 1) * QB],
                        in0=ps[hl * 64:(hl + 1) * 64, :],
                        scalar1=bk_sb[hl * 64:(hl + 1) * 64, mt:mt + 1],
                        scalar2=None,
                        op0=mybir.AluOpType.add,
                    )

        def q_proj(p):
            """Q^T for queries of block p; bias via gpsimd tensor_scalar."""
            for mt in range(n_mt):
                ps = psD.tile([128, QB], fp32, tag="proj")
                for kti in range(n_dt):
                    nc.tensor.matmul(
                        out=ps,
                        lhsT=wq_sb[:, kti, mt * 128:(mt + 1) * 128],
                        rhs=xq_t[p][:, kti, :],
                        start=(kti == 0),
                        stop=(kti == n_dt - 1),
                    )
                for hl in range(2):
                    h = 2 * mt + hl
                    nc.vector.tensor_scalar(
                        out=QT[h][0:64, p * QB:(p + 1) * QB],
                        in0=ps[hl * 64:(hl + 1) * 64, :],
                        scalar1=bq_sb[hl * 64:(hl + 1) * 64, mt:mt + 1],
                        scalar2=None,
                        op0=mybir.AluOpType.add,
                    )

        def v_proj(p):
            """V rows [p*QB, (p+1)*QB) -> V_sb tiles 4p..4p+3 (q-major), with
            the key-padding mask folded into rows and ones-column."""
            for mi in range(sub):
                mt = sub * p + mi
                ps = psD.tile([128, dpc], fp32, tag="proj")
                for kti in range(n_dt):
                    nc.tensor.matmul(
                        out=ps,
                        lhsT=xv_t[p][:, kti, mi * 128:(mi + 1) * 128],
                        rhs=wv_sb[:, kti, :],
                        start=(kti == 0),
                        stop=(kti == n_dt - 1),
                    )
                nc.vector.tensor_scalar(
                    out=V_sb[mt][:, :, 0:64],
                    in0=ps.rearrange("p (h e) -> p h e", h=hpc),
                    scalar1=vmask_sb[:, mt:mt + 1],
                    scalar2=None,
                    op0=mybir.AluOpType.mult,
                )
                nc.vector.memset(V_sb[mt][:, :, 64:65], 1.0)
                nc.vector.tensor_scalar_mul(
                    out=V_sb[mt][:, :, 64:65],
                    in0=V_sb[mt][:, :, 64:65],
                    scalar1=vmask_sb[:, mt:mt + 1],
                )

        def attn_scores(qb, h, pts):
            """S^T then exp for head h, q block qb; paired-k-tile PSUM tiles."""
            last_kt = sub * qb + sub - 1
            for kp in range((last_kt + 1 + 1) // 2):
                k0, k1 = 2 * kp, 2 * kp + 1
                s_ps = psS.tile([128, 2 * QB], fp32, tag="s")
                o0 = max(0, 128 * (k0 - sub * qb))
                nc.tensor.matmul(
                    out=s_ps[:, o0:QB],
                    lhsT=KTt[h][:, k0 * KT:(k0 + 1) * KT],
                    rhs=QT[h][:, qb * QB + o0:(qb + 1) * QB],
                    start=True, stop=True,
                )
                o1 = max(0, 128 * (k1 - sub * qb))
                nc.tensor.matmul(
                    out=s_ps[:, QB + o1:2 * QB],
                    lhsT=KTt[h][:, k1 * KT:(k1 + 1) * KT],
                    rhs=QT[h][:, qb * QB + o1:(qb + 1) * QB],
                    start=True, stop=True,
                )
                pt = ppool.tile([128, 2 * QB], bf16, tag="pt")
                if o1 == 0:
                    # both halves fully written: one exp over the pair
                    nc.scalar.activation(
                        out=pt[:, o0:2 * QB],
                        in_=s_ps[:, o0:2 * QB],
                        func=mybir.ActivationFunctionType.Exp,
                    )
                else:
                    # diagonal pair: skip the unwritten [QB, QB+o1) gap
                    nc.scalar.activation(
                        out=pt[:, o0:QB],
                        in_=s_ps[:, o0:QB],
                        func=mybir.ActivationFunctionType.Exp,
                    )
                    nc.scalar.activation(
                        out=pt[:, QB + o1:2 * QB],
                        in_=s_ps[:, QB + o1:2 * QB],
                        func=mybir.ActivationFunctionType.Exp,
                    )
                for kk, oo in ((k0, o0), (k1, o1)):
                    v = kk - sub * qb
                    if v >= 0:
                        base = (kk - 2 * kp) * QB
                        # NOT gpsimd: CollectiveCompute blocks the Pool queue
                        # for its full duration, stalling anything behind it
                        nc.vector.tensor_mul(
                            out=pt[:, base + 128 * v:base + 128 * (v + 1)],
                            in0=pt[:, base + 128 * v:base + 128 * (v + 1)],
                            in1=tri_sb,
                        )
                pts.append(pt)

        def attn_pv(qb, hp, pts2):
            """PV + normalize + combined transpose for head pair hp."""
            for qs in range(sub):
                cq = cqpool.tile([128, 128], bf16, tag="cq")
                for hl in range(2):
                    h = 2 * hp + hl
                    ctx_ps = psC.tile([128, 65], fp32, tag="ctx")
                    nkt = sub * qb + qs + 1
                    for kti in range(nkt):
                        pt = pts2[hl][kti // 2]
                        base = (kti % 2) * QB
                        nc.tensor.matmul(
                            out=ctx_ps,
                            lhsT=pt[:, base + qs * 128:base + (qs + 1) * 128],
                            rhs=V_sb[kti][:, h, :],
                            start=(kti == 0),
                            stop=(kti == nkt - 1),
                        )
                    rcp = spool.tile([128, 1], fp32, tag="rcp")
                    nc.vector.reciprocal(out=rcp, in_=ctx_ps[:, 64:65])
                    nc.vector.tensor_scalar(
                        out=cq[:, hl * 64:(hl + 1) * 64],
                        in0=ctx_ps[:, 0:64],
                        scalar1=rcp,
                        scalar2=None,
                        op0=mybir.AluOpType.mult,
                    )
                # transpose output rides the psD "proj" ring (no spare bank)
                tr_ps = psD.tile([128, 128], bf16, tag="proj")
                nc.tensor.transpose(out=tr_ps, in_=cq, identity=ident_sb)
                nc.vector.tensor_copy(
                    out=ctxT[hp][:, qb * QB + qs * 128:qb * QB + (qs + 1) * 128],
                    in_=tr_ps,
                )

        def out_proj(qb, mis=range(4)):
            """Row-parallel partial out for q block qb (row chunks `mis`)."""
            for mi in mis:
                mt = sub * qb + mi
                po = opool.tile([128, d], bf16, tag="po")
                for oc in range(d // QB):
                    ps = psD.tile([128, QB], fp32, tag="proj")
                    for t in range(n_mt):
                        nc.tensor.matmul(
                            out=ps,
                            lhsT=ctxT[t][:, mt * 128:(mt + 1) * 128],
                            rhs=wo_sb[t][:, oc * QB:(oc + 1) * QB],
                            start=(t == 0),
                            stop=(t == n_mt - 1),
                        )
                    nc.vector.tensor_add(
                        out=po[:, oc * QB:(oc + 1) * QB],
                        in0=ps,
                        in1=bo4_sb[:, oc * QB:(oc + 1) * QB],
                    )
                nc.sync.dma_start(
                    out=partials[qb][mi * 128:(mi + 1) * 128, :], in_=po)

        def reduce_scatter(qb):
            # the NEFF verifier forbids collectives writing IO tensors, so
            # bounce rs_out -> SBUF -> out_ext on the idle sync queue
            nc.gpsimd.collective_compute(
                "ReduceScatter",
                mybir.AluOpType.add,
                replica_groups=groups,
                ins=[partials[qb][:, :]],
                outs=[rs_out[qb, :, :]],
            )
            osb = osbp.tile([128, d], bf16, tag="osb")
            nc.sync.dma_start(out=osb, in_=rs_out[qb, :, :])
            nc.sync.dma_start(out=out_ext[qb, :, :], in_=osb)

        # ---- the pipeline ----
        # Per-head-pair scores->PV keeps at most ~16 pt tiles live; the
        # next panel's projections and the PREVIOUS block's out-proj fill
        # PE bubbles while exp catches up on the fresh scores.  qb0 starts
        # on K+Q only; v_proj(0) fills the first scores' exp latency (PV
        # doesn't need V until after the first exps complete).
        k_proj(0)
        q_proj(0)
        for qb in range(n_qb):
            pts = [[] for _ in range(hpc)]
            for h in range(hpc):
                attn_scores(qb, h, pts[h])
            if qb == 0:
                v_proj(0)
            else:
                out_proj(qb - 1)
                reduce_scatter(qb - 1)
            if qb + 1 < n_qb:
                k_proj(qb + 1)
            attn_pv(qb, 0, (pts[0], pts[1]))
            if qb + 1 < n_qb:
                v_proj(qb + 1)
            attn_pv(qb, 1, (pts[2], pts[3]))
            if qb + 1 < n_qb:
                q_proj(qb + 1)
        out_proj(n_qb - 1)
        reduce_scatter(n_qb - 1)

    nc.compile()
    return nc


def prep_core_inputs(inputs, core, seq=SEQ, d=D, hpc=HPC):
    """Host-side shard/layout prep for one core.  Pure layout + dtype work."""
    b, g = divmod(core, G)
    dpc = hpc * DH
    sl = slice(g * dpc, (g + 1) * dpc)
    n_dt = d // 128
    n_kt = seq // KT
    n_mt = dpc // 128

    def pack_x(x):
        # [128, n_dt, seq]: [p, kti, s] = x[b][s, kti*128+p]
        return np.ascontiguousarray(
            np.asarray(x[b]).T.reshape(n_dt, 128, seq).transpose(1, 0, 2)
        ).astype(BF16)

    def pack_w(w, scale=1.0):
        # [128, n_dt, dpc]: [p, kti, j] = w[sl][j, kti*128+p]
        wt = (np.asarray(w)[sl, :].T * scale).astype(np.float32)
        return np.ascontiguousarray(
            wt.reshape(n_dt, 128, dpc).transpose(1, 0, 2)).astype(BF16)

    def pack_bias(bias, scale=1.0):
        return np.ascontiguousarray(
            (np.asarray(bias)[sl] * scale).reshape(n_mt, 128).T
        ).astype(np.float32)

    kpm = np.asarray(inputs["key_padding_mask"])
    vmask = np.ascontiguousarray(
        (~kpm[b]).astype(np.float32).reshape(n_kt, 128).T)

    tri = (np.arange(KT)[:, None] <= np.arange(KT)[None, :]).astype(BF16)

    wo = np.asarray(inputs["wo"]).astype(np.float32)
    bv = np.asarray(inputs["bv"]).astype(np.float32)
    bo = np.asarray(inputs["bo"]).astype(np.float32)
    bo4 = bo / G + bv[sl] @ wo[:, sl].T

    s = 1.0 / np.sqrt(DH)
    return {
        "xq": pack_x(inputs["q_input"]),
        "xk": pack_x(inputs["k_input"]),
        "xv": pack_x(inputs["v_input"]),
        "wq": pack_w(inputs["wq"], scale=s),
        "wk": pack_w(inputs["wk"]),
        "wv": pack_w(inputs["wv"]),
        "woT": np.ascontiguousarray(wo.T[sl, :]).astype(BF16),
        "tri": tri,
        "ident": np.eye(128, dtype=BF16),
        "bo4": np.tile(bo4, (128, 1)).astype(np.float32),
        "bq": pack_bias(inputs["bq"], scale=s),
        "bk": pack_bias(inputs["bk"]),
        "vmask": vmask,
    }


def assemble_output(core_outs, seq=SEQ, d=D):
    """core_outs[4b+j] has shape [n_qb, 128, d]: chunk c holds reduced
    output rows QB*c + 128*j of batch b."""
    out = np.empty((B, seq, d), dtype=np.float32)
    n_qb = seq // QB
    for core in range(NCORES):
        b, j = divmod(core, G)
        co = np.asarray(core_outs[core]).astype(np.float32).reshape(n_qb, 128, d)
        for c in range(n_qb):
            r0 = QB * c + 128 * j
            out[b, r0:r0 + 128, :] = co[c]
    return out


_CACHED_NC = None


def _get_nc():
    global _CACHED_NC
    if _CACHED_NC is None:
        _CACHED_NC = build_program()
    return _CACHED_NC


def kernel(**inputs) -> np.ndarray:
    nc = _get_nc()
    in_maps = [prep_core_inputs(inputs, core) for core in range(NCORES)]
    res = run_bass_kernel_spmd(nc, in_maps, core_ids=list(range(NCORES)))
    return assemble_output([res.results[c]["out"] for c in range(NCORES)])


if __name__ == "__main__":
    nc = build_program()
    print("program built ok")
